# revision 10
# baseline (speedup 1.0000x reference)
"""Trainium2 Bass kernel for nn_DiscreteFlow (checkerboard discrete flow).

Data-parallel over 8 NeuronCores (128 samples each). Per core, samples are
processed in NSG sample-groups of S=8.

Layouts (per core):
  rows stored "slot-major": slot q in 0..65; q=0 -> row 63 (wrap halo),
  q=1+r -> row r (r=0..63), q=65 -> row 0 (wrap halo).
  x    SBUF fp16 [64 part(col), (s 8, q 66)] per sample-group tile
  h1   SBUF fp32 [128 part(colInGrp 8 x ci 16), (G 11, s 8, q 66)]
  h2   SBUF fp32 [96 part(colInBlk 6 x ci 16), (b 11, s 8, q 66)]
  x6i  SBUF fp16 [60 part(w 10 x dr 3 x pair 2), (G 11, s 8, pi 2, j 32)]
  mirror DRAM fp16 [72 (col -2..69), NSG, 8, 66]  (masked x, halo'd cols)

Convs (all stationaries built host-side as Toeplitz blocks):
  conv1: per group G (11 overlapping 8-col groups, outcols 6G-1..6G+6):
         1 matmul  K=60 (w,dr,Whi/Wlo fp16 pair)  M=128 (oc8 x co16)  N=512
  conv2: per group G: 3 matmuls (kr)  K=128 (c8 x ci16)  M=96 (oc6 x co16)
         rhs = h1 row-taps via slot slices, accumulated in PSUM
  conv3: 13 chunk matmuls  K=96 (c6 x ci16)  M=96 (krow3 x oc32), 2 M-halves
         stage-2 krow shift-add on DVE, sign via ACT with A-site big-bias
"""
import os
import sys

sys.path.insert(0, "/opt/trn_rl_repo")

import numpy as np
from contextlib import ExitStack

import concourse.bass as bass
import concourse.bacc as bacc
import concourse.tile as tile
import concourse.bass_utils as bass_utils
from concourse import mybir
from concourse.tile import TileContext

F32 = mybir.dt.float32
F32R = mybir.dt.float32r
F16 = mybir.dt.float16

L = 64
N_LAYERS = 4
B = 1024
N_CORES = 8
S = 8                     # samples per group
BIG = 1.0e5

# conv3 chunk lists: (mb, h2-block) pairs
CH_MB0 = [0, 1, 2, 3, 4, 5, 10]
CH_MB1 = [5, 6, 7, 8, 9, 10]
N_CHUNKS = len(CH_MB0) + len(CH_MB1)  # 13

# dtype for conv2/conv3 matmuls: float32 (exact, 4 cyc/row) or
# float32r (tf32-grade, 1 cyc/row at N>=256).  See module docstring.
DT2_NAME = os.environ.get("BASS_DT2", "float32r")


def _dt2():
    return F32R if DT2_NAME == "float32r" else F32


# ----------------------------------------------------------------------------
# Host-side input preparation
# ----------------------------------------------------------------------------

def _slot_expand(xrows):
    """[..., 64] rows -> [..., 66] slots with wrap halos."""
    out = np.concatenate(
        [xrows[..., 63:64], xrows, xrows[..., 0:1]], axis=-1)
    return out


def host_prep(z, W0, b0, W1, b1, W2, b2, nsg):
    """Build per-core input dicts. z: (B, 4096) fp32."""
    z = np.asarray(z, dtype=np.float32)
    W0 = np.asarray(W0, np.float32); W1 = np.asarray(W1, np.float32)
    W2 = np.asarray(W2, np.float32)
    b0 = np.asarray(b0, np.float32); b1 = np.asarray(b1, np.float32)
    b2 = np.asarray(b2, np.float32).reshape(N_LAYERS, -1)[:, 0]

    ncores = z.shape[0] // (nsg * S)
    # ---- weight-derived stationaries (shared across cores) ----
    W0h = W0.astype(np.float16)
    W0l = (W0 - W0h.astype(np.float32)).astype(np.float16)
    T1 = np.zeros((N_LAYERS, 60, 128), np.float16)
    for l in range(N_LAYERS):
        for w in range(10):
            for dr in range(3):
                for p in range(2):
                    k = w * 6 + dr * 2 + p
                    Wp = W0h if p == 0 else W0l
                    for oc in range(8):
                        kc = w - oc
                        if 0 <= kc <= 2:
                            T1[l, k, oc * 16:(oc + 1) * 16] = Wp[l, dr, kc, 0, :]

    T2 = np.zeros((N_LAYERS, 3, 128, 96), np.float32)
    for l in range(N_LAYERS):
        for kr in range(3):
            for c in range(8):
                for oc in range(6):
                    kc = c - oc
                    if 0 <= kc <= 2:
                        T2[l, kr, c * 16:(c + 1) * 16, oc * 16:(oc + 1) * 16] = \
                            W1[l, kr, kc, :, :]

    # conv3: chunk index -> (mb, block) ; M index = krow*32 + oc
    T3 = np.zeros((N_LAYERS, N_CHUNKS, 96, 96), np.float32)
    chunk_of = {}
    for i, b_ in enumerate(CH_MB0):
        chunk_of[(0, b_)] = i
    for i, b_ in enumerate(CH_MB1):
        chunk_of[(1, b_)] = len(CH_MB0) + i
    for l in range(N_LAYERS):
        for mb in range(2):
            for oc in range(32):
                outcol = mb * 32 + oc
                for kc in range(3):
                    tt = outcol + kc - 1
                    if tt == -1:
                        bc, c6 = 10, 3          # col 63 lives in block 10 slot 3
                    elif tt == 64:
                        bc, c6 = 10, 4          # col-0 dup in block 10 slot 4
                    elif tt <= 59:
                        bc, c6 = tt // 6, tt % 6
                    else:
                        bc, c6 = 10, tt - 60
                    ci_idx = chunk_of[(mb, bc)]
                    for krow in range(3):
                        T3[l, ci_idx, c6 * 16:(c6 + 1) * 16, krow * 32 + oc] = \
                            W2[l, krow, kc, :, 0]

    b0d = np.zeros((N_LAYERS, 128), np.float32)
    b1d = np.zeros((N_LAYERS, 96), np.float32)
    for l in range(N_LAYERS):
        b0d[l] = np.tile(b0[l], 8)
        b1d[l] = np.tile(b1[l], 6)

    # sign bias: [l, col, pi]  (pi = row parity)
    bsd = np.zeros((N_LAYERS, 64, 2), np.float32)
    for l in range(N_LAYERS):
        pA = 0 if l % 2 == 0 else 1
        for c in range(64):
            for pi in range(2):
                isA = ((pi + c) % 2) == pA
                bsd[l, c, pi] = b2[l] + (BIG if isA else 0.0)

    # x-masks for NEXT layer's A-parity: [pA, col, s, q66] fp16
    rows = np.arange(66) - 1  # slot q -> row
    rows[0] = 63
    rows[65] = 0
    xmd = np.zeros((64, 2, S, 66), np.float16)
    for pA in range(2):
        m = ((rows[None, :] + np.arange(64)[:, None]) % 2 == pA)
        xmd[:, pA] = m[:, None, :].astype(np.float16)

    shared = dict(
        T1d=T1,
        T2d=np.ascontiguousarray(
            T2.transpose(0, 2, 1, 3).reshape(N_LAYERS, 128, 288)),
        T3d=np.ascontiguousarray(
            T3.transpose(0, 2, 1, 3).reshape(N_LAYERS, 96, N_CHUNKS * 96)),
        b0d=b0d[:, :, None], b1d=b1d[:, :, None], bsd=bsd, xmd=xmd)

    # ---- per-core x0 / mirror0 ----
    in_maps = []
    zg = z.reshape(ncores, nsg, S, 64, 64)      # [core, sg, s, row, col]
    pA0 = 0  # layer 0 partition 'even'
    for core in range(ncores):
        xc = zg[core]                            # [nsg, s, row, col]
        x0 = np.ascontiguousarray(
            _slot_expand(xc.transpose(3, 0, 1, 2))).astype(np.float16)
        # x0: [col 64, nsg, s, q 66]
        # site (r,c) is A iff (r+c)%2 == pA0
        mrc = ((np.arange(64)[:, None] + np.arange(64)[None, :]) % 2 == pA0)
        xm = xc * mrc[None, None, :, :]          # [nsg, s, r, c]
        xmt = xm.transpose(3, 0, 1, 2)           # [col, nsg, s, r]
        xms = _slot_expand(xmt)                  # [col, nsg, s, q]
        # mirror: [72, nsg, dr 3, s, t 64]; [cs, n, dr, s, t] = slot (dr+t)
        mir = np.zeros((72, nsg, 3, S, 64), np.float16)
        full = np.zeros((72, nsg, S, 66), np.float16)
        full[2:66] = xms
        full[0:2] = xms[62:64]
        full[66:70] = xms[0:4]
        for dr in range(3):
            mir[:, :, dr] = full[:, :, :, dr:dr + 64]
        im = dict(x0=x0, mirror0=mir)
        im.update(shared)
        in_maps.append(im)
    return in_maps


# ----------------------------------------------------------------------------
# Device kernel
# ----------------------------------------------------------------------------

def build_nc(nsg, dt2):
    nc = bacc.Bacc("TRN2", target_bir_lowering=False, debug=False,
                   detect_race_conditions=False)

    ap = {}
    def din(name, shape, dt):
        ap[name] = nc.dram_tensor(name, list(shape), dt, kind="ExternalInput").ap()

    din("x0", (64, nsg, S, 66), F16)
    din("mirror0", (72, nsg, 3, S, 64), F16)
    din("T1d", (N_LAYERS, 60, 128), F16)
    din("T2d", (N_LAYERS, 128, 288), dt2)
    din("T3d", (N_LAYERS, 96, N_CHUNKS * 96), dt2)
    din("b0d", (N_LAYERS, 128, 1), F32)
    din("b1d", (N_LAYERS, 96, 1), F32)
    din("bsd", (N_LAYERS, 64, 2), F32)
    din("xmd", (64, 2, S, 66), F16)
    out_ap = nc.dram_tensor("xout", [64, nsg, S, 64], F32,
                            kind="ExternalOutput").ap()

    FQ = S * 66          # free size per (G|sg) block: s x q
    AF = mybir.ActivationFunctionType

    with TileContext(nc) as tc, ExitStack() as ctx:
        stat = ctx.enter_context(tc.tile_pool(name="stat", bufs=2))
        const = ctx.enter_context(tc.tile_pool(name="const", bufs=1))
        xpool = ctx.enter_context(tc.tile_pool(name="x", bufs=1))
        hpool = ctx.enter_context(tc.tile_pool(name="h", bufs=2))
        wpool = ctx.enter_context(tc.tile_pool(name="w", bufs=2))
        ypool = ctx.enter_context(tc.tile_pool(name="y", bufs=3))
        opool = ctx.enter_context(tc.tile_pool(name="o", bufs=2))
        pscv = ctx.enter_context(tc.tile_pool(name="pscv", bufs=2, space="PSUM"))
        psu = ctx.enter_context(tc.tile_pool(name="psu", bufs=2, space="PSUM"))
        drm = ctx.enter_context(tc.tile_pool(name="drm", bufs=1, space="DRAM"))

        # masks resident
        xmsk = const.tile([64, 2 * S * 66], F16, tag="xmsk")
        nc.sync.dma_start(xmsk[:], ap["xmd"][:, :, :, :])

        # dedicated (never-recycled) buffers for tiles that DMAs read from
        xm3s = []
        ocs = []
        for i in range(2):
            xm3_t = const.tile([64, 3 * S * 64], F16, tag=f"xm3b{i}")
            xm3s.append(xm3_t)
            oc_t = const.tile([64, S * 64], F32, tag=f"ocb{i}")
            ocs.append(oc_t)

        # x tiles (one per sample group), loaded once
        xts = []
        for sg in range(nsg):
            xt = xpool.tile([64, FQ], F16, tag=f"xt{sg}")
            nc.sync.dma_start(xt[:], ap["x0"][:, sg, :, :])
            xts.append(xt)

        # internal DRAM mirrors for layers 1..3
        mirrors = [None] * N_LAYERS
        for l in range(1, N_LAYERS):
            mir_t = drm.tile([72, nsg * 3 * S * 64], F16, tag=f"mir{l}")
            mirrors[l] = mir_t

        for l in range(N_LAYERS):
            dt2_l = dt2
            T1s = stat.tile([60, 128], F16, tag="T1s")
            nc.sync.dma_start(T1s[:], ap["T1d"][l, :, :])
            T2s = stat.tile([128, 288], dt2, tag="T2s")
            nc.sync.dma_start(T2s[:], ap["T2d"][l, :, :])
            T3s = stat.tile([96, N_CHUNKS * 96], dt2, tag="T3s")
            nc.sync.dma_start(T3s[:], ap["T3d"][l, :, :])
            b0s = stat.tile([128, 1], F32, tag="b0s")
            nc.sync.dma_start(b0s[:], ap["b0d"][l, :, :])
            b1s = stat.tile([96, 1], F32, tag="b1s")
            nc.sync.dma_start(b1s[:], ap["b1d"][l, :, :])
            bss = stat.tile([64, 2], F32, tag="bss")
            nc.sync.dma_start(bss[:], ap["bsd"][l, :, :])

            for sg in range(nsg):
                xt = xts[sg]
                # ---- build x6i from mirror (DRAM) ----
                x6 = wpool.tile([60, 11 * S * 64], F16, tag="x6")
                if os.environ.get("BASS_SIM_MEMSET"):
                    nc.gpsimd.memset(x6[:], 0.0)
                x6v = x6[:].rearrange("(w k) (g st) -> w k g st", k=6, g=11)
                if l == 0:
                    mrd = ap["mirror0"]
                    # mrd dims: [72, nsg, 3, s, 64]
                    def msrc(gs, wl, dr):
                        v = mrd[:, sg, dr, :, :].rearrange(
                            "(gg k) s t -> gg k s t", k=6)
                        return v[gs:gs + 11, 0:wl, :, :] \
                            .rearrange("gg k s t -> k gg (s t)")
                else:
                    mt = mirrors[l]
                    mrv = mt[:].rearrange(
                        "(gg k) (ns dr s t) -> gg k ns dr s t",
                        k=6, ns=nsg, dr=3, s=S)
                    def msrc(gs, wl, dr):
                        return mrv[gs:gs + 11, 0:wl, sg, dr, :, :] \
                            .rearrange("gg k s t -> k gg (s t)")
                for dr in range(3):
                    for p in range(2):
                        dst = x6v[:, dr * 2 + p, :, :]
                        # w 0..5 from group G, w 6..9 from group G+1
                        nc.sync.dma_start(dst[0:6], msrc(0, 6, dr))
                        nc.sync.dma_start(dst[6:10], msrc(1, 4, dr))

                # ---- conv1: 11 matmuls in 4 waves of <=3 ----
                h1 = hpool.tile([128, 11 * FQ], dt2, tag="h1")
                h1v = h1[:].rearrange("p (g s q) -> p g s q", g=11, s=S)
                waves = [(0, 3), (3, 3), (6, 3), (9, 2)]
                for w0, wn in waves:
                    ps = pscv.tile([128, 3 * 512], F32, tag="cv")
                    for gi in range(wn):
                        G = w0 + gi
                        rhs = x6[:].rearrange(
                            "p (g s t) -> p g s t", g=11, s=S)[:, G, :, :] \
                            .rearrange("p s (j pi) -> p pi s j", pi=2)
                        nc.tensor.matmul(
                            ps[:, gi * 512:(gi + 1) * 512], T1s[:], rhs,
                            start=True, stop=True)
                    # evacuate wave: tanh(psum + b0), per G (ACT APs are <=3D)
                    for gi in range(wn):
                        G = w0 + gi
                        src = ps[:, gi * 512:(gi + 1) * 512].rearrange(
                            "p (pi s j) -> p pi s j", pi=2, s=S)
                        dst = h1v[:, G, :, 1:65].rearrange(
                            "p s (j pi) -> p pi s j", pi=2)
                        nc.scalar.activation(dst, src, AF.Tanh, bias=b0s[:])
                # halo fills
                nc.vector.tensor_copy(h1v[:, :, :, 0:1], h1v[:, :, :, 64:65])
                nc.vector.tensor_copy(h1v[:, :, :, 65:66], h1v[:, :, :, 1:2])

                # ---- conv2: per group, 3 kr matmuls ----
                h2 = hpool.tile([96, 11 * FQ], dt2, tag="h2")
                h2v = h2[:].rearrange("p (g s q) -> p g s q", g=11, s=S)
                for w0, wn in waves:
                    ps = pscv.tile([96, 3 * 512], F32, tag="cv")
                    for gi in range(wn):
                        G = w0 + gi
                        for kr in range(3):
                            rhs = h1v[:, G, :, kr:kr + 64].rearrange(
                                "p s (j pi) -> p pi s j", pi=2)
                            nc.tensor.matmul(
                                ps[:, gi * 512:(gi + 1) * 512],
                                T2s[:, kr * 96:(kr + 1) * 96], rhs,
                                start=(kr == 0), stop=(kr == 2))
                    for gi in range(wn):
                        G = w0 + gi
                        src = ps[:, gi * 512:(gi + 1) * 512].rearrange(
                            "p (pi s j) -> p pi s j", pi=2, s=S)
                        dst = h2v[:, G, :, 1:65].rearrange(
                            "p s (j pi) -> p pi s j", pi=2)
                        nc.scalar.activation(dst, src, AF.Tanh, bias=b1s[:])

                # ---- conv3 ----
                y = ypool.tile([64, S * 64], F32, tag="y")
                yv = y[:].rearrange("p (s q) -> p s q", s=S)
                for mb in range(2):
                    chunks = CH_MB0 if mb == 0 else CH_MB1
                    coff = 0 if mb == 0 else len(CH_MB0)
                    ps3 = psu.tile([96, 512], F32, tag="u")
                    for i, b_ in enumerate(chunks):
                        ci_ = coff + i
                        rhs = h2v[:, b_, :, 1:65]
                        nc.tensor.matmul(
                            ps3[:], T3s[:, ci_ * 96:(ci_ + 1) * 96], rhs,
                            start=(i == 0), stop=(i == len(chunks) - 1))
                    u = ps3[:].rearrange("p (s q) -> p s q", s=S)
                    yo = yv[mb * 32:(mb + 1) * 32, :, :]
                    # y[r] = u0[r-1] + u1[r] + u2[r+1]
                    # (only one PSUM operand allowed per DVE op)
                    nc.vector.tensor_copy(yo[:, :, :], u[32:64, :, :])
                    nc.vector.tensor_add(yo[:, :, 1:64],
                                         yo[:, :, 1:64], u[0:32, :, 0:63])
                    nc.vector.tensor_add(yo[:, :, 0:1],
                                         yo[:, :, 0:1], u[0:32, :, 63:64])
                    nc.vector.tensor_add(yo[:, :, 0:63],
                                         yo[:, :, 0:63], u[64:96, :, 1:64])
                    nc.vector.tensor_add(yo[:, :, 63:64],
                                         yo[:, :, 63:64], u[64:96, :, 0:1])

                # ---- sign + update ----
                sm = ypool.tile([64, S * 64], F16, tag="sm")
                smv = sm[:].rearrange("p (s q) -> p s q", s=S)
                for pi in range(2):
                    nc.scalar.activation(
                        smv[:, :, pi::2], yv[:, :, pi::2], AF.Sign,
                        bias=bss[:, pi:pi + 1])
                xtv = xt[:].rearrange("p (s q) -> p s q", s=S)
                nc.vector.tensor_mul(xtv[:, :, 1:65], xtv[:, :, 1:65],
                                     smv[:, :, :])
                nc.vector.tensor_copy(xtv[:, :, 0:1], xtv[:, :, 64:65])
                nc.vector.tensor_copy(xtv[:, :, 65:66], xtv[:, :, 1:2])

                if l < N_LAYERS - 1:
                    # masked mirror write-back, pre-tripled by dr
                    pA = 0 if (l + 1) % 2 == 0 else 1
                    xm3 = xm3s[sg % 2]
                    x3v = xm3[:].rearrange("p (dr s t) -> p dr s t", dr=3, s=S)
                    mskv = xmsk[:].rearrange("p (a s q) -> p a s q", a=2, s=S)
                    for dr in range(3):
                        nc.vector.tensor_mul(x3v[:, dr, :, :],
                                             xtv[:, :, dr:dr + 64],
                                             mskv[:, pA, :, dr:dr + 64])
                    mt = mirrors[l + 1]
                    mrv = mt[:].rearrange(
                        "m (ns f) -> m ns f", ns=nsg)
                    nc.sync.dma_start(mrv[2:66, sg, :], xm3[:, :])
                    nc.sync.dma_start(mrv[0:2, sg, :], xm3[62:64, :])
                    nc.sync.dma_start(mrv[66:70, sg, :], xm3[0:4, :])
                else:
                    oc = ocs[sg % 2]
                    ocv = oc[:].rearrange("p (s q) -> p s q", s=S)
                    nc.vector.tensor_copy(ocv[:, :, :], xtv[:, :, 1:65])
                    nc.sync.dma_start(out_ap[:, sg, :, :], ocv[:, :, :])

    nc.compile()
    return nc


# ----------------------------------------------------------------------------
# Entry point
# ----------------------------------------------------------------------------

_NC_CACHE = {}


def _get_nc(nsg, dt2name):
    key = (nsg, dt2name)
    if key not in _NC_CACHE:
        _NC_CACHE[key] = build_nc(nsg, F32R if dt2name == "float32r" else F32)
    return _NC_CACHE[key]


def unshard(results, nsg):
    outs = []
    for r in results:
        o = r["xout"]                       # [64, nsg, S, 64] f32
        outs.append(np.transpose(o, (1, 2, 3, 0)).reshape(nsg * S, 4096))
    return np.concatenate(outs, axis=0)


def kernel(z, W0, b0, W1, b1, W2, b2):
    z = np.asarray(z, np.float32)
    ncores = N_CORES
    nsg = z.shape[0] // (ncores * S)
    nc = _get_nc(nsg, DT2_NAME)
    in_maps = host_prep(z, W0, b0, W1, b1, W2, b2, nsg)
    res = bass_utils.run_bass_kernel_spmd(
        nc, in_maps, core_ids=list(range(ncores)))
    return unshard(res.results, nsg).astype(np.float32)
